# revision 14
# baseline (speedup 1.0000x reference)
"""Trainium2 Bass kernel for batched per-frequency steering-matrix application.

Computes Y[b,t,k,n] = sum_m X[b,t,k,m] * (U_real + i*U_imag)[pid[b],k,m,n]
as complex64, distributed data-parallel over batch across 8 NeuronCores.

Device strategy per core (2 batch samples):
  - Host gathers U[pid] and builds block-diagonal weight tiles packing 8
    frequency bins into a full 128-wide contraction, with (n, real/imag)
    interleaved into 256 output columns so one bf16 matmul
    [K=128, M=100(t), N=256] directly produces complex64 memory layout.
  - Host pre-transposes X to [b, (bin_sub, mic), group, t] so the matmul
    stationary operand DMAs in densely with no on-chip transpose.
  - Output is written in a fully-dense per-(b, t-tile, 13-group-block)
    layout (1.33 MB contiguous chunks, 13 KB per partition run) to keep
    HWDGE descriptor-generation off the critical path; host unshuffles.
  - Device: DMA in -> matmul -> PSUM->SBUF copy (vector/scalar, batched
    4 groups per copy) -> DMA out.
"""

import sys

for _p in ("/opt/trn_rl_repo", "/root/.axon_site/_ro/trn_rl_repo"):
    if _p not in sys.path:
        sys.path.append(_p)

import numpy as np
import ml_dtypes


def _install_ntff_hook_shim():
    """The image's antenv lacks axon_hooks; synthesize it so trace=True can
    capture NTFF profiles via /opt/axon/libaxon_pjrt.so."""
    try:
        import antenv.axon_hooks  # noqa: F401
        return
    except ImportError:
        pass
    import types
    import contextlib
    import ctypes

    mod = types.ModuleType("antenv.axon_hooks")
    mod._hook = None

    def set_axon_ntff_profile_hook(h):
        mod._hook = h

    def get_axon_ntff_profile_hook():
        return mod._hook

    mod.set_axon_ntff_profile_hook = set_axon_ntff_profile_hook
    mod.get_axon_ntff_profile_hook = get_axon_ntff_profile_hook
    sys.modules["antenv.axon_hooks"] = mod
    try:
        import antenv

        antenv.axon_hooks = mod
    except ImportError:
        pass

    so_path = "/opt/axon/libaxon_pjrt.so"
    try:
        lib = ctypes.CDLL(so_path)
        if not hasattr(lib, "axon_start_nrt_profile"):
            return
        lib.axon_start_nrt_profile.argtypes = [
            ctypes.POINTER(ctypes.c_int64),
            ctypes.c_size_t,
        ]
        lib.axon_start_nrt_profile.restype = ctypes.c_int64
        lib.axon_stop_nrt_profile.argtypes = [ctypes.c_char_p]
        lib.axon_stop_nrt_profile.restype = ctypes.c_int64
    except OSError:
        return

    @contextlib.contextmanager
    def _hook(output_dir, device_ids):
        import jax

        jax.devices()
        if device_ids:
            ids = (ctypes.c_int64 * len(device_ids))(*device_ids)
            rc = lib.axon_start_nrt_profile(ids, len(device_ids))
        else:
            rc = lib.axon_start_nrt_profile(None, 0)
        if rc != 0:
            raise RuntimeError(f"axon_start_nrt_profile rc={rc}")
        try:
            yield
        finally:
            n = lib.axon_stop_nrt_profile(str(output_dir).encode())
            print(f"ntff profile: {n} file(s) written to {output_dir}", file=sys.stderr)

    mod._hook = _hook


_install_ntff_hook_shim()

# ---- problem constants (hardcoded per spec) ----
NDOA, B, T, NBIN, NMIC = 36, 16, 400, 513, 16
NCORES = 8
BLOC = B // NCORES        # 2 batch samples per core
NG = 65                   # groups of 8 bins; 513 padded to 520
NBIN_PAD = NG * 8
TT = 4                    # t tiles per batch row
TSZ = T // TT             # 100
NC2 = 2 * NMIC            # 32 interleaved (n, c) per bin
NFREE = 8 * NC2           # 256 matmul output columns per group
NSB = 5                   # sblocks per batch row
GPS = NG // NSB           # 13 groups per sblock

_cache = {}


def _build(trace=False):
    """Build the Bass graph (one SPMD program, same for all cores)."""
    import concourse.bass as bass
    import concourse.mybir as mybir
    import concourse.tile as tile
    from concourse import bacc

    nc = bacc.Bacc(None, target_bir_lowering=False)

    x = nc.declare_dram_parameter("x", [BLOC, 128, NG, T], mybir.dt.bfloat16, isOutput=False)
    bd = nc.declare_dram_parameter("bd", [BLOC, 128, NG, NFREE], mybir.dt.bfloat16, isOutput=False)
    # dense output: [b, sb, t, gi*nc], bf16 (host upcasts + reorders)
    out = nc.declare_dram_parameter(
        "out", [BLOC, NSB, T, GPS * NFREE], mybir.dt.bfloat16, isOutput=True
    )

    # t tiles of 128 partitions (+16 tail) so out-DMA descriptors spread
    # across all 16 SDMA engines
    TSPLIT = [(0, 128), (128, 128), (256, 128), (384, 16)]

    with tile.TileContext(nc) as tc:
        with (
            tc.tile_pool(name="xp", bufs=4) as xp,
            tc.tile_pool(name="bdp", bufs=4) as bdp,
            tc.tile_pool(name="stage", bufs=8) as stage,
            tc.tile_pool(name="psum", bufs=4, space="PSUM") as psum,
        ):
            nout = 0
            # loop (b, sb, tt): each input chunk retires after its 4 t-tiles,
            # so loads self-pace through the span instead of front-loading
            for b in range(BLOC):
                for sb in range(NSB):
                    g0, g1 = sb * GPS, (sb + 1) * GPS
                    # chunked loads on the SWDGE (gpsimd) path, off the
                    # HWDGE rings that carry the output stream
                    xt = xp.tile([128, GPS * T], mybir.dt.bfloat16, tag="xb")
                    bt = bdp.tile([128, GPS * NFREE], mybir.dt.bfloat16, tag="bdb")
                    if b == 0 and sb == 0:
                        # kernel-start critical path: the very first quads'
                        # inputs go via the (idle, FIFO) HWDGE rings in small
                        # pieces so compute starts in ~2us; SWDGE prefetch of
                        # later sblocks can't starve them
                        for gg0, gg1, eng in (
                            (0, 4, nc.sync),
                            (4, 8, nc.scalar),
                            (8, GPS, nc.gpsimd),
                        ):
                            eng.dma_start(
                                bt[:, gg0 * NFREE : gg1 * NFREE],
                                bd[b, :, g0 + gg0 : g0 + gg1, :],
                            )
                            eng.dma_start(
                                xt[:, gg0 * T : gg1 * T],
                                x[b, :, g0 + gg0 : g0 + gg1, :],
                            )
                    else:
                        gh = 7
                        nc.gpsimd.dma_start(bt[:, : gh * NFREE], bd[b, :, g0 : g0 + gh, :])
                        nc.gpsimd.dma_start(xt[:, : gh * T], x[b, :, g0 : g0 + gh, :])
                        nc.gpsimd.dma_start(bt[:, gh * NFREE :], bd[b, :, g0 + gh : g1, :])
                        nc.gpsimd.dma_start(xt[:, gh * T :], x[b, :, g0 + gh : g1, :])
                    for t0, tlen in TSPLIT:
                        st = stage.tile([tlen, GPS * NFREE], mybir.dt.bfloat16, tag="st")
                        # 13 groups = 3 quads + 1 remainder, batched per PSUM tile
                        nq = 0
                        for q0 in range(0, GPS, 4):
                            qn = min(4, GPS - q0)
                            ps = psum.tile([tlen, 4 * NFREE], mybir.dt.float32, tag="ps")
                            for gi in range(q0, q0 + qn):
                                lhsT = xt[:, gi * T + t0 : gi * T + t0 + tlen]
                                rhs = bt[:, gi * NFREE : (gi + 1) * NFREE]
                                nc.tensor.matmul(
                                    ps[:, (gi - q0) * NFREE : (gi - q0 + 1) * NFREE],
                                    lhsT,
                                    rhs,
                                    start=True,
                                    stop=True,
                                )
                            dst = st[:, q0 * NFREE : (q0 + qn) * NFREE]
                            src = ps[:, : qn * NFREE]
                            # casting copy f32 PSUM -> bf16 SBUF
                            if nq in (1, 2):
                                nc.scalar.copy(dst, src)
                            else:
                                nc.vector.tensor_copy(dst, src)
                            nq = (nq + 1) % 4
                        # alternate output DMAs across both HWDGE rings
                        if nout % 2 == 0:
                            nc.sync.dma_start(out[b, sb, t0 : t0 + tlen, :], st[:])
                        else:
                            nc.scalar.dma_start(out[b, sb, t0 : t0 + tlen, :], st[:])
                        nout += 1
    nc.compile()
    return nc


def _get_nc():
    if "nc" not in _cache:
        _cache["nc"] = _build()
    return _cache["nc"]


def _host_prep(X, pid, U_real, U_imag):
    X = np.asarray(X, np.float32)
    pid = np.asarray(pid).astype(np.int64)
    U_real = np.asarray(U_real, np.float32)
    U_imag = np.asarray(U_imag, np.float32)

    # gather + stack real/imag: [B, NBIN, M, N, 2]
    Ug = np.stack([U_real[pid], U_imag[pid]], axis=-1)
    Ug_p = np.zeros((B, NBIN_PAD, NMIC, NMIC, 2), np.float32)
    Ug_p[:, :NBIN] = Ug
    # block-diagonal: BD[b, g, ks*16+m, ks*32 + n*2 + c]
    Ugr = Ug_p.reshape(B, NG, 8, NMIC, NC2)
    BDf = np.zeros((B, NG, 8, NMIC, 8, NC2), np.float32)
    for ks in range(8):
        BDf[:, :, ks, :, ks, :] = Ugr[:, :, ks]
    # partition-major [b, p, g, nc] for dense DMA
    BDp = np.ascontiguousarray(
        BDf.reshape(B, NG, 128, NFREE).transpose(0, 2, 1, 3)
    ).astype(ml_dtypes.bfloat16)

    # X: [b,t,k,m] -> [b,k,m,t] -> pad -> [b, p=(ks,m), g, t]
    Xt = X.transpose(0, 2, 3, 1)
    Xp_ = np.zeros((B, NBIN_PAD, NMIC, T), np.float32)
    Xp_[:, :NBIN] = Xt
    Xp_ = Xp_.reshape(B, NG, 8, NMIC, T).transpose(0, 2, 3, 1, 4)
    Xp = np.ascontiguousarray(Xp_.reshape(B, 128, NG, T)).astype(ml_dtypes.bfloat16)
    return Xp, BDp


def _unshuffle(full):
    """[B, NSB, T, GPS*NFREE] bf16 -> complex64 [B, T, NBIN, NMIC]"""
    a = full.reshape(B, NSB, T, GPS * NFREE)
    # -> [b, t, sb, gi*nc], upcast to f32
    a = np.ascontiguousarray(a.transpose(0, 2, 1, 3), dtype=np.float32)
    c = a.reshape(B, T, NG * 128, 2).view(np.complex64)[..., 0]
    return np.ascontiguousarray(c.reshape(B, T, NBIN_PAD, NMIC)[:, :, :NBIN])


def _run(in_maps, trace=False):
    from concourse.bass_utils import run_bass_kernel_spmd

    nc = _get_nc()
    res = run_bass_kernel_spmd(nc, in_maps, core_ids=list(range(NCORES)), trace=trace)
    return res


def kernel(X, pid, U_real, U_imag, _trace=False, _return_results=False):
    Xp, BDp = _host_prep(X, pid, U_real, U_imag)
    in_maps = [
        {
            "x": np.ascontiguousarray(Xp[i * BLOC : (i + 1) * BLOC]),
            "bd": np.ascontiguousarray(BDp[i * BLOC : (i + 1) * BLOC]),
        }
        for i in range(NCORES)
    ]
    res = _run(in_maps, trace=_trace)
    full = np.concatenate([r["out"] for r in res.results], axis=0)
    out = _unshuffle(full)
    if _return_results:
        return out, res
    return out
